# revision 38
# baseline (speedup 1.0000x reference)
"""MoE top-1 routed expert FFN (8 experts) on 8 Trainium2 NeuronCores.

Strategy: expert parallelism. Core e holds expert e's weights. The host
computes the token->expert permutation (top-1 dispatch is just a gather),
ships each core its tokens transposed (tokens on the matmul free dim),
and the device runs the whole FFN in transposed token space:

    hT = gelu_tanh(w1_tile.T @ xT + b1)        (per 128-wide ff tile)
    yT = sum_ff w2_tile.T @ hT + b2            (accumulated in PSUM)

so w1 ([D, FF]) and w2 ([FF, D]) act as PE stationary operands in their
natural layouts and no on-device transpose is needed. The host scatters
each core's yT back into the full output (tokens are disjoint across
experts, so the source's all-reduce degenerates to a scatter).

Matmul operands are fp16 (fast weight load + full-rate PE, ~5e-4 rel err)
with fp32 PSUM accumulation; the fp16 moving width (>=256) keeps each
LDWEIGHTS hidden under the previous matmul, so the warm stream runs at the
N/2.4GHz+2.5ns issue roofline. The schedule fights the three overheads the
roofline leaves:

- HAM cold clock: the PE runs at 1.2 GHz until it has been busy ~3.4us
  straight. A burst of dummy matmuls starts the busy streak at the first
  possible instruction and bridges until the first x/w DMAs land (queue
  start latency ~1.5us, ~300 GB/s aggregate shared fairly across queues,
  so the critical bytes ride at the front of the sync/gpsimd FIFOs).
- DMA-delivery margin: packs are consumed once, in a phase-1 stream that
  runs mm1+gelu for BOTH chunks plus mm2 for chunk0 (18 matmuls / 2us per
  pack vs ~1.3-1.6us delivery), with chunk1's h parked in SBUF; phase 2
  replays chunk1's mm2 from SBUF — a 16us pure-PE tail with zero DMA
  dependence under which chunk0's writeback hides. This also keeps PSUM
  at 8 banks (2 ph + 6 py). mm2(c0,ff) issues after mm1 of ff+1, so each
  gelu has ~2us of PE cover before the in-order queue needs its result.
- Tail: yT stages through SBUF as fp16 (half the writeback bytes), the
  final chunk's bias-adds run on ACT and DVE in parallel right as each
  PSUM d-tile finishes, output DMAs alternate two queues, and the tile
  pools are merged 7->4 to shrink the pool-teardown barrier chain. (The
  remaining ~6us of epilogue — the 256-semaphore wipe split across
  engines plus engine-exit chain — is fixed framework cost.)
"""

import os

import numpy as np

import concourse.mybir as mybir
import concourse.tile as tile
from concourse import bacc, bass_utils

N_CORES = 8
D = 768
FF = 3072
KD = D // 128  # 6
KF = FF // 128  # 24
NPACK = KF  # one ff-tile of (w1 slab | w2 tile) per DMA pack
NWARM = 32  # dummy matmuls: arm the HAM clock gate + bridge to first data

_compiled = {}


def _maybe_trace():
    """Enable NTFF tracing only when MOE_TRACE=1 and the axon profile hook
    can be installed. The graded path never sets the env var."""
    if not os.environ.get("MOE_TRACE"):
        return False
    try:
        import sys
        import types

        if "antenv.axon_hooks" not in sys.modules:
            mod = types.ModuleType("antenv.axon_hooks")
            _h = [None]
            mod.set_axon_ntff_profile_hook = lambda h: _h.__setitem__(0, h)
            mod.get_axon_ntff_profile_hook = lambda: _h[0]
            sys.modules["antenv.axon_hooks"] = mod
            from trn_agent_boot.trn_boot import _ntff_profile_via_ctypes

            mod.set_axon_ntff_profile_hook(
                _ntff_profile_via_ctypes("/opt/axon/libaxon_pjrt.so")
            )
        return True
    except Exception:
        return False


def _build(chunks):
    """Build + compile the per-core FFN kernel for token chunk sizes `chunks`."""
    C = sum(chunks)
    f32 = mybir.dt.float32
    f16 = mybir.dt.float16
    gelu = mybir.ActivationFunctionType.Gelu_apprx_tanh
    ident = mybir.ActivationFunctionType.Identity

    nc = bacc.Bacc("TRN2", target_bir_lowering=False, debug=False, num_devices=N_CORES)
    # xp[p, h*(3C) + q] grouped [half][chunk][k%3][c] so each (half, chunk)
    # quarter is one contiguous DMA
    xp_d = nc.dram_tensor("xp", [128, KD * C], f16, kind="ExternalInput").ap()
    # wp[ff]: [w1h(ff) | w2(ff)], each half a [128, 768] lhsT slab
    wp_d = nc.dram_tensor("wp", [NPACK, 128, 2 * D], f16, kind="ExternalInput").ap()
    # bp[:, :KF] = b1 tiles, bp[:, KF:KF+KD] = b2 tiles
    bp_d = nc.dram_tensor("bp", [128, KF + KD], f32, kind="ExternalInput").ap()
    yT_d = nc.dram_tensor("yT", [D, C], f16, kind="ExternalOutput").ap()

    offs = [sum(chunks[:j]) for j in range(len(chunks))]
    n_ch = len(chunks)
    # x quarter layout offsets within a half tile: [chunk0 3*C0 | chunk1 3*C1]
    qoff = [3 * sum(chunks[:j]) for j in range(n_ch)]

    with tile.TileContext(nc) as tc:
        with (
            tc.tile_pool(name="spool", bufs=1) as spool,
            tc.tile_pool(name="hypool", bufs=6) as hypool,
            tc.tile_pool(name="phpool", bufs=2, space="PSUM") as phpool,
            tc.tile_pool(name="pypool", bufs=1, space="PSUM") as pypool,
        ):
            # PE warmup: dummy matmuls with no DMA dependency start the HAM
            # busy streak at the first possible instruction and keep the PE
            # busy while the first input DMAs are in flight. The memset runs
            # on gpsimd, whose queue is free before its first DMA issue.
            warm_w = spool.tile([128, 128], f16, tag="warm")
            nc.gpsimd.memset(warm_w[:], 0.0)
            warm_ps = phpool.tile([128, chunks[0]], f32, tag="ph", name="warm_ps")
            for _ in range(NWARM):
                nc.tensor.matmul(
                    warm_ps[:, :128], warm_w[:], warm_w[:], start=True, stop=True
                )

            # input DMAs across three DGE queues, in consumption order:
            # x(chunk0 halves) + pack0 unblock the first mm1/mm2 steps, the
            # remaining packs stream round-robin; chunk1's x is consumed a
            # full chunk pass later so it trails the first packs
            x_sb = [
                spool.tile([128, 3 * C], f16, tag=f"x{j}", name=f"x{j}")
                for j in range(2)
            ]
            w_sb = [
                spool.tile([128, 2 * D], f16, tag=f"wp{i}", name=f"wp{i}")
                for i in range(NPACK)
            ]
            b_sb = spool.tile([128, KF + KD], f32, tag="b")

            C0 = chunks[0]
            # Aggregate DMA bandwidth (~300 GB/s, ~1.5us queue start
            # latency) is shared fairly across active queues, so priority =
            # FIFO position. Transfers queue in exact consumption order of
            # the interleaved schedule: x(c0)+pack0.w1h first, then x(c1),
            # pack0.w2, pack1, and the pack stream.
            # sync: x(h0,c0), x(h1,c0), pack0 w2-half, even packs
            nc.sync.dma_start(x_sb[0][:, : 3 * C0], xp_d[:, : 3 * C0])
            # split at the k3 boundary: mm1(c0) k3 unblocks on one slab
            nc.sync.dma_start(x_sb[1][:, :C0], xp_d[:, 3 * C : 3 * C + C0])
            nc.sync.dma_start(
                x_sb[1][:, C0 : 3 * C0], xp_d[:, 3 * C + C0 : 3 * C + 3 * C0]
            )
            nc.sync.dma_start(w_sb[0][:, D:], wp_d[0, :, D:])
            # gpsimd: pack0 w1-half, x(h0,c1), pack1 (split in halves so
            # mm1 of ff=1 unblocks a half-pack earlier), odd packs
            nc.gpsimd.dma_start(w_sb[0][:, :D], wp_d[0, :, :D])
            if n_ch > 1:
                nc.gpsimd.dma_start(
                    x_sb[0][:, 3 * C0 :], xp_d[:, 3 * C0 : 3 * C]
                )
            nc.gpsimd.dma_start(w_sb[1][:, :D], wp_d[1, :, :D])
            nc.gpsimd.dma_start(w_sb[1][:, D:], wp_d[1, :, D:])
            # scalar (free until the first gelu): biases, then x(h1,c1) —
            # a third stream for one front transfer, behind the table load
            nc.scalar.dma_start(b_sb[:], bp_d)
            if n_ch > 1:
                nc.scalar.dma_start(
                    x_sb[1][:, 3 * C0 :], xp_d[:, 3 * C + 3 * C0 :]
                )
            qrr = [nc.sync, nc.gpsimd]
            for i in range(2, NPACK):
                qrr[i % 2].dma_start(w_sb[i][:], wp_d[i, :, :])

            # Two-phase schedule. Phase 1 streams the packs once: per ff,
            # mm1(c0)+gelu, mm1(c1)+gelu (h kept in SBUF), mm2(c0) — 18 PE
            # matmuls per pack, so DMA delivery (~1.3us/pack) has ~0.7us of
            # slack per step. Phase 2 replays mm2(c1) from the stored h
            # tiles: a 16us pure-PE stream with zero DMA dependence, under
            # which chunk0's writeback hides. PSUM: 2 ph + 6 py banks = 8.
            # mm2(c0,ff) is issued after mm1 of ff+1, so each gelu has two
            # matmul groups of cover before the in-order PE needs it.
            py = {}

            def py_tile(ci, d):
                Cc = chunks[ci]
                py[(ci, d)] = pypool.tile(
                    [128, Cc], f32, tag=f"py{d}", name=f"py{d}_{ci}"
                )

            h_tiles = {}

            def mm1(ci, ff):
                Cc = chunks[ci]
                wt = w_sb[ff]
                ph = phpool.tile([128, Cc], f32, tag="ph", name=f"ph_{ci}_{ff}")
                for k in range(KD):
                    xoff = qoff[ci] + (k % 3) * Cc
                    nc.tensor.matmul(
                        ph[:],
                        wt[:, k * 128 : (k + 1) * 128],
                        x_sb[k // 3][:, xoff : xoff + Cc],
                        start=(k == 0),
                        stop=(k == KD - 1),
                    )
                if ci == 0:
                    h_sb = hypool.tile([128, Cc], f16, tag="h", name=f"h0_{ff}")
                else:
                    # chunk1's h persists until phase 2 replays it
                    h_sb = spool.tile([128, Cc], f16, tag=f"h1_{ff}")
                nc.scalar.activation(
                    h_sb[:], ph[:], gelu, bias=b_sb[:, ff : ff + 1], scale=1.0
                )
                h_tiles[(ci, ff)] = h_sb

            def mm2(ci, ff):
                wt = w_sb[ff]
                h_sb = h_tiles.pop((ci, ff))
                if ff == 0:
                    for d in range(KD):
                        py_tile(ci, d)
                for d in range(KD):
                    nc.tensor.matmul(
                        py[(ci, d)][:],
                        wt[:, D + d * 128 : D + (d + 1) * 128],
                        h_sb[:],
                        start=(ff == 0),
                        stop=(ff == KF - 1),
                    )

            def y_out(ci, d, on_act):
                Cc, c0 = chunks[ci], offs[ci]
                y_sb = hypool.tile([128, Cc], f16, tag="y", name=f"y_{ci}_{d}")
                b2ap = b_sb[:, KF + d : KF + d + 1]
                if on_act:
                    nc.scalar.activation(y_sb[:], py[(ci, d)][:], ident, bias=b2ap)
                else:
                    nc.vector.tensor_scalar_add(y_sb[:], py[(ci, d)][:], b2ap)
                qrr[d % 2].dma_start(
                    yT_d[d * 128 : (d + 1) * 128, c0 : c0 + Cc], y_sb[:]
                )

            # phase 1
            for ff in range(KF):
                for ci in range(n_ch):
                    mm1(ci, ff)
                if ff > 0:
                    mm2(0, ff - 1)
            mm2(0, KF - 1)
            # chunk0's writeback: DVE/ACT both free once the gelus are done;
            # everything overlaps phase 2's PE stream
            for d in range(KD):
                y_out(0, d, on_act=(d % 2 == 1))
            # phase 2, d-major: each yT d-tile finishes its 24-ff
            # accumulation ~2.7us before the next, so its bias-add + DMA
            # overlap the remaining matmuls; only the last tile drains
            # after the PE stream ends
            for ci in range(1, n_ch):
                for d in range(KD):
                    py_tile(ci, d)
                    for ff in range(KF):
                        nc.tensor.matmul(
                            py[(ci, d)][:],
                            w_sb[ff][:, D + d * 128 : D + (d + 1) * 128],
                            h_tiles[(ci, ff)][:],
                            start=(ff == 0),
                            stop=(ff == KF - 1),
                        )
                    y_out(ci, d, on_act=(d % 2 == 0))
    nc.compile()
    return nc


def _get_compiled(chunks):
    key = tuple(chunks)
    if key not in _compiled:
        _compiled[key] = _build(list(key))
    return _compiled[key]


def kernel(inputs, dispatch_order, w1, b1, w2, b2):
    x = np.asarray(inputs, dtype=np.float32)
    B, S, Dm = x.shape
    T = B * S
    xf = x.reshape(T, Dm)
    disp = np.asarray(dispatch_order).astype(np.int64)
    w1 = np.asarray(w1, dtype=np.float32)
    b1 = np.asarray(b1, dtype=np.float32)
    w2 = np.asarray(w2, dtype=np.float32)
    b2 = np.asarray(b2, dtype=np.float32)
    E = w1.shape[0]

    counts = np.bincount(disp, minlength=E)
    cmax = max(int(counts.max()), 16)
    # token capacity per core: near-equal chunks of <=512 (PSUM bank limit
    # for fp32 accumulation), multiples of 16, as small as cmax allows
    C = -(-cmax // 16) * 16
    n_chunks = -(-C // 512)
    chunks = []
    rem = C
    for j in range(n_chunks):
        c = -(-(rem // (n_chunks - j)) // 16) * 16
        chunks.append(c)
        rem -= c
    chunks.sort(reverse=True)

    order = np.argsort(disp, kind="stable")
    starts = np.concatenate([[0], np.cumsum(counts)])

    in_maps = []
    for e in range(E):
        ids = order[starts[e] : starts[e + 1]]
        xe = np.zeros((C, Dm), dtype=np.float32)
        xe[: len(ids)] = xf[ids]
        # xq[p, h*(3C) + chunk-major cols]: group by (half, chunk) so each
        # quarter ships as one contiguous DMA
        xk = xe.reshape(C, KD, 128)  # [c, k, p]
        parts = []
        for h in range(2):
            for ci in range(len(chunks)):
                c0 = sum(chunks[:ci])
                cc = chunks[ci]
                # [p, 3, cc] -> [p, 3*cc]
                parts.append(
                    xk[c0 : c0 + cc, 3 * h : 3 * h + 3, :]
                    .transpose(2, 1, 0)
                    .reshape(128, 3 * cc)
                )
        xp = np.concatenate(parts, axis=1)
        # w1 in lhsT slab layout: w1h[ff][p, k*128+c] = w1[k*128+p, ff*128+c]
        w1h = (
            w1[e]
            .reshape(KD, 128, KF, 128)
            .transpose(2, 1, 0, 3)
            .reshape(KF, 128, KD * 128)
        )
        w2t = w2[e].reshape(KF, 128, D)
        wp = np.concatenate([w1h, w2t], axis=2)
        bp = np.concatenate(
            [b1[e].reshape(KF, 128).T, b2[e].reshape(KD, 128).T], axis=1
        )
        in_maps.append(
            {
                "xp": np.ascontiguousarray(xp).astype(np.float16),
                "wp": np.ascontiguousarray(wp).astype(np.float16),
                "bp": np.ascontiguousarray(bp),
            }
        )

    nc = _get_compiled(chunks)
    res = None
    for attempt in range(3):
        try:
            res = bass_utils.run_bass_kernel_spmd(
                nc, in_maps, core_ids=list(range(N_CORES)), trace=_maybe_trace()
            )
            break
        except Exception:
            # transient runtime/tunnel hiccups: retry a couple of times
            if attempt == 2:
                raise
            import time

            time.sleep(2.0)
    if res.exec_time_ns is not None:
        print(f"HW exec time: {res.exec_time_ns} ns")
        if res.instructions_and_trace is not None:
            print(f"trace: {res.instructions_and_trace[1]}")

    out = np.zeros((T, Dm), dtype=np.float32)
    for e in range(E):
        ids = order[starts[e] : starts[e + 1]]
        yT = res.results[e]["yT"]
        out[ids] = yT[:, : len(ids)].T.astype(np.float32)
    return out.reshape(B, S, Dm)
